# revision 6
# baseline (speedup 1.0000x reference)
"""TRN2 Bass kernel for nn_ConceptEmbeddingConceptPred.

Computes y = concat([einsum('bjd,ijd->bi', x, W_emb) + b_loo,
                     einsum('bjd,hjd->bh', x, W_full) + b_full], axis=1)
where W_emb is the leave-one-out scatter-embedding of W_loo (zero diagonal).

Flattened, this is a (4096 x 16384) @ (16384 x 136) GEMM.

Distribution: contraction(k)-parallel over the 8 cores — core c owns
concepts j in [16c, 16c+16) (k-slice of 2048). Each core computes a full
(136, 4096) partial product; partials are summed on the host (cheap),
bias added, transposed, concatenated.

v7 dataflow (fp8e3 x stream, fp16 weights, fp32 PSUM accumulate):
  - x is cast to float8_e3m4 on the host (1.3% rel rms quantization,
    halves DMA to 8.4 MB/core). The PE runs the mixed-dtype matmul
    (fp16 stationary x fp8e3 moving) at the full 1 col/cycle rate.
  - x lives in SBUF as 8 kt-pair tiles [128, 2, 4096] (all resident),
    DMA'd as one 512 KB transfer per (pair, round-half) for line rate;
    the first pair is split finer so the first matmul starts early.
  - phase-separated passes (no PE tile-mode-switch drains): loo round 0
    (64 back-to-back M=128 matmuls), loo round 1, then one full-probe
    pass (M=8) in 9 balanced ~455-col chunks: col groups q0/q32/q64 get
    exactly 3 concurrent streams per k-tile (quadrant 3 is unusable).
  - no ACT-engine ops (avoids the preamble ACT_TABLE_LOAD that delays
    the scalar queue's first DMA); all PSUM evacuation on vector.
  - warmup matmuls rotate 3 PSUM banks so they pipeline instead of
    serializing on same-bank WAW deps.
"""

import sys

for _p in ("/opt/trn_rl_repo",):
    if _p not in sys.path:
        sys.path.append(_p)

import numpy as np
import ml_dtypes
import concourse.bacc as bacc
import concourse.mybir as mybir
import concourse.tile as tile
from concourse.bass_utils import run_bass_kernel_spmd

dt = mybir.dt

B, C, D, H = 4096, 128, 128, 8
NCORES = 8
JPC = C // NCORES  # 16 concept (= k) tiles per core
KPC = JPC * D  # 2048 contraction elements per core
NPAIR = JPC // 2  # kt-pair x tiles
BCHUNK = 512  # loo batch chunk (fp32 PSUM bank limit)
RCHUNK = 2048  # batch cols per loo round
NR = 2  # loo rounds
NWARM = 14  # pipelined warmup matmuls (PE p-state ramp during DMA fill)
NFCH = 9  # full-probe chunks: 3 col groups x 3 streams each, balanced
FEDGE = [round(B * c / NFCH) for c in range(NFCH + 1)]  # chunk edges
NFB = 3  # full-probe PSUM banks (chunk c -> bank c//3, group c%3)

_nc_cache = None


def _build():
    global _nc_cache
    if _nc_cache is not None:
        return _nc_cache

    nc = bacc.Bacc(
        "TRN2", target_bir_lowering=False, debug=False, num_devices=NCORES
    )
    # x as kt-pair-major 4D so one DMA covers a [128, 2, cols] region
    xt_d = nc.dram_tensor(
        "x_t4", (NPAIR, 128, 2, B), dt.float8e3, kind="ExternalInput"
    ).ap()
    wl_d = nc.dram_tensor(
        "w_loo_t", (D, JPC, C), dt.float16, kind="ExternalInput"
    ).ap()
    wf_d = nc.dram_tensor(
        "w_full_t", (D, JPC, H), dt.float16, kind="ExternalInput"
    ).ap()
    yl_d = nc.dram_tensor("y_loo_t", (C, B), dt.float16, kind="ExternalOutput").ap()
    # full-probe outputs in packed col-group layout: bank w rows
    # [32g : 32g+8] hold chunk c = 3w + g over cols [FEDGE[c], FEDGE[c+1])
    yf_d = nc.dram_tensor(
        "y_full_p", (NFB, 128, BCHUNK), dt.float16, kind="ExternalOutput"
    ).ap()

    with tile.TileContext(nc) as tc:
        with (
            tc.tile_pool(name="wpool", bufs=1) as wpool,
            tc.tile_pool(name="xpool", bufs=8) as xpool,
            tc.tile_pool(name="ylpool", bufs=2) as ylpool,
            tc.tile_pool(name="yfpool", bufs=3) as yfpool,
            tc.tile_pool(name="psl", bufs=5, space="PSUM") as psl,
            tc.tile_pool(name="psf", bufs=3, space="PSUM") as psf,
        ):
            wl = wpool.tile([D, JPC, C], dt.float16)
            wf = wpool.tile([D, JPC, H], dt.float16)
            # kt0 loo weights ride sync ahead of x; the rest on scalar
            nc.sync.dma_start(wl[:, 0:1, :], wl_d[:, 0:1, :])
            nc.scalar.dma_start(wf[:], wf_d[:])
            nc.scalar.dma_start(wl[:, 1:, :], wl_d[:, 1:, :])

            # warmup: rotate 3 PSUM banks so consecutive matmuls pipeline
            # (same-bank WAW would serialize at full drain latency)
            warm_w = wpool.tile([128, 128], dt.float16)
            warm_x = wpool.tile([128, 128], dt.float16)
            nc.vector.memset(warm_w[:], 0.25)
            nc.vector.memset(warm_x[:], 0.25)
            warm_ps = [
                psf.tile([128, BCHUNK], dt.float32, tag="accf", name=f"warm{i}")
                for i in range(3)
            ]
            for i in range(NWARM):
                nc.tensor.matmul(
                    warm_ps[i % 3][:, :128],
                    warm_w[:],
                    warm_x[:],
                    start=True,
                    stop=True,
                )

            # x: 8 pair tiles [128, 2, B]; round-0 cols first (pair0 split
            # finer for the earliest possible first matmul), then round 1.
            xts = [
                xpool.tile([128, 2, B], dt.float8e3, tag="xn", name=f"xp_{p}")
                for p in range(NPAIR)
            ]
            nc.sync.dma_start(xts[0][:, 0, 0:1024], xt_d[0][:, 0, 0:1024])
            nc.sync.dma_start(xts[0][:, 0, 1024:RCHUNK], xt_d[0][:, 0, 1024:RCHUNK])
            nc.sync.dma_start(xts[0][:, 1, 0:RCHUNK], xt_d[0][:, 1, 0:RCHUNK])
            for p in range(1, NPAIR):
                eng = nc.scalar if p % 2 == 1 else nc.sync
                eng.dma_start(xts[p][:, :, 0:RCHUNK], xt_d[p][:, :, 0:RCHUNK])
            for p in range(NPAIR):
                eng = nc.sync if p % 2 == 1 else nc.scalar
                eng.dma_start(xts[p][:, :, RCHUNK:], xt_d[p][:, :, RCHUNK:])

            def xsl(kt, a, b):
                return xts[kt // 2][:, kt % 2, a:b]

            # loo rounds: 64 back-to-back M=128 matmuls each
            for r in range(NR):
                accs = [
                    psl.tile(
                        [C, BCHUNK], dt.float32, tag="accl", name=f"accl{r}_{c}"
                    )
                    for c in range(4)
                ]
                for kt in range(JPC):
                    for c in range(4):
                        b0 = r * RCHUNK + c * BCHUNK
                        nc.tensor.matmul(
                            accs[c][:],
                            wl[:, kt, :],
                            xsl(kt, b0, b0 + BCHUNK),
                            start=(kt == 0),
                            stop=(kt == JPC - 1),
                        )
                yl_sb = ylpool.tile([C, RCHUNK], dt.float16, tag="yl")
                for c in range(4):
                    nc.vector.tensor_copy(
                        yl_sb[:, c * BCHUNK : (c + 1) * BCHUNK], accs[c][:]
                    )
                oeng = nc.sync if r == 0 else nc.scalar
                oeng.dma_start(yl_d[:, r * RCHUNK : (r + 1) * RCHUNK], yl_sb[:])

            # full-probe pass: 9 balanced chunks over col groups q0/q32/q64
            fbanks = [
                psf.tile([128, BCHUNK], dt.float32, tag="accf", name=f"fb{w}")
                for w in range(NFB)
            ]
            for kt in range(JPC):
                for c in range(NFCH):
                    w, g = divmod(c, NFB)
                    a, b = FEDGE[c], FEDGE[c + 1]
                    nc.tensor.matmul(
                        fbanks[w][32 * g : 32 * g + H, : b - a],
                        wf[:, kt, :],
                        xsl(kt, a, b),
                        start=(kt == 0),
                        stop=(kt == JPC - 1),
                    )
            for w in range(NFB):
                yf_sb = yfpool.tile([128, BCHUNK], dt.float16, tag="yf")
                nc.vector.tensor_copy(yf_sb[:], fbanks[w][:])
                oeng = nc.sync if w % 2 == 0 else nc.scalar
                oeng.dma_start(yf_d[w], yf_sb[:])

    nc.compile()
    _nc_cache = nc
    return nc


def _embed_loo_weights(W_loo):
    # probe i sees concepts j != i; scatter into (C, C, D) with zero row at j=i
    I = np.arange(C)[:, None]
    J = np.arange(C)[None, :]
    src = np.clip(J - (J > I).astype(np.int64), 0, C - 2)  # (C, C)
    W_emb = np.take_along_axis(W_loo, src[:, :, None], axis=1)  # (C, C, D)
    return W_emb * (J != I)[:, :, None].astype(W_loo.dtype)


def _prep_in_maps(x, W_loo, W_full):
    x32 = np.asarray(x, dtype=np.float32)
    # (C, D, B): each core's (JPC, D, B) k-slice is contiguous; then pack
    # kt-pair-major (NPAIR, 128, 2, B) to match the 4D DRAM layout
    xt_all = np.ascontiguousarray(x32.transpose(1, 2, 0)).astype(
        ml_dtypes.float8_e3m4
    )
    W_emb = _embed_loo_weights(np.asarray(W_loo, dtype=np.float32))
    W_full = np.asarray(W_full, dtype=np.float32)
    in_maps = []
    for c in range(NCORES):
        jsl = slice(c * JPC, (c + 1) * JPC)
        xt_c = (
            xt_all[jsl]
            .reshape(NPAIR, 2, D, B)
            .transpose(0, 2, 1, 3)
        )
        xt_c = np.ascontiguousarray(xt_c)
        # stationary layouts: [d, kt, out] so K (=d) is the partition dim
        wl_c = np.ascontiguousarray(
            W_emb[:, jsl, :].transpose(2, 1, 0).astype(np.float16)
        )
        wf_c = np.ascontiguousarray(
            W_full[:, jsl, :].transpose(2, 1, 0).astype(np.float16)
        )
        in_maps.append({"x_t4": xt_c, "w_loo_t": wl_c, "w_full_t": wf_c})
    return in_maps


def _assemble(results, b_loo, b_full):
    y_loo_t = np.zeros((C, B), np.float64)
    y_full_t = np.zeros((H, B), np.float64)
    for res in results:
        y_loo_t += res["y_loo_t"]
        yf_p = res["y_full_p"]  # (NFB, 128, BCHUNK) packed col groups
        for c in range(NFCH):
            w, g = divmod(c, NFB)
            a, b = FEDGE[c], FEDGE[c + 1]
            y_full_t[:, a:b] += yf_p[w, 32 * g : 32 * g + H, : b - a]
    y_loo = (y_loo_t.T + np.asarray(b_loo, np.float64)[None, :]).astype(np.float32)
    y_full = (y_full_t.T + np.asarray(b_full, np.float64)[None, :]).astype(np.float32)
    return np.concatenate([y_loo, y_full], axis=1)


def run_spmd(x, W_loo, b_loo, W_full, b_full, trace=False):
    nc = _build()
    in_maps = _prep_in_maps(x, W_loo, W_full)
    res = run_bass_kernel_spmd(
        nc, in_maps, core_ids=list(range(NCORES)), trace=trace
    )
    return _assemble(res.results, b_loo, b_full), res


def kernel(x, W_loo, b_loo, W_full, b_full):
    out, _ = run_spmd(x, W_loo, b_loo, W_full, b_full)
    return out
